# revision 53
# baseline (speedup 1.0000x reference)
"""Trainium2 Bass kernel for nn_GCLSTM (B=512, T=32, H=300, 10 neighbors).

Sharding: T is split across 8 cores (4 timesteps each). The reference's
scan carry (`con`) is a pure function of the per-step input, so every
timestep's cells() output can be computed independently; each core only
additionally computes cells() for its t0-1 block to get `lastcon`.
The flat-reshape softmax scramble mixes the whole batch at fixed t, so
keeping B intact per core makes it core-local.

The t=0 step (core 0 only) uses con0 directly instead of
relu(wp@[con;lastcon]+bp); this is folded into the uniform SPMD program
by giving core 0 identity/zero weights for its tau=0 con1 matmul and a
leaky-relu eviction with per-core alpha (1.0 on core 0 -> passthrough).
"""
import os
import sys

for _p in ("/opt/trn_rl_repo", "/root/.axon_site", "/root/.axon_site/_ro/trn_rl_repo",
           "/root/.axon_site/_ro/pypackages"):
    if os.path.isdir(_p) and _p not in sys.path:
        sys.path.append(_p)

import numpy as np
import ml_dtypes
from contextlib import ExitStack

import concourse.bass as bass
import concourse.tile as tile
from concourse.tile_rust import add_dep_helper
from concourse import bacc, mybir
from concourse import bass_utils
from concourse.bass_interp import get_hw_module

BF16 = mybir.dt.bfloat16
F32 = mybir.dt.float32
FP8 = mybir.dt.float8e4
NPBF = ml_dtypes.bfloat16
NPF8 = ml_dtypes.float8_e4m3
AF = mybir.ActivationFunctionType
ALU = mybir.AluOpType
DR = mybir.MatmulPerfMode.DoubleRow

B, T, H = 512, 32, 300
NCORES = 8
TLOC = T // NCORES            # 4 timesteps per core
R = TLOC * B                  # 2048 rows per core (tau*512 + b)
NK = 10                       # neighbors
HC = [(0, 128), (128, 128), (256, 44)]   # H=300 chunking (offset, size)
NRT = R // 128                # 16 row-tiles of 128
# tau' cell blocks: 5 per core (t0-1 .. t0+3), processed in pairs
TPAIRS = [(0, 1), (2, 3), (4,)]


_BUILD_CACHE = {}


def _build():
    nc = bacc.Bacc("TRN2", target_bir_lowering=False, debug=False,
                   enable_asserts=False, num_devices=NCORES)

    def din(name, shape, dt):
        return nc.dram_tensor(name, shape, dt, kind="ExternalInput").ap()

    # --- per-core data inputs ---
    xT = din("xT", [NK, 12, 5 * B], BF16)        # [k, i(11)+ones, tau'*512+b]
    xrow = din("xrow", [R, 11], F32)             # wext(10) fusc(1), host-folded
    exT = din("exT", [5, R], BF16)               # extras features + ones row
    # --- weights (replicated; wpT0/bp0/alpha differ on core 0) ---
    wihT = din("wihT", [NK, 12, 900], BF16)      # per-k lstm weights + bias row
    wtT = din("wtT", [5, 900], BF16)             # target-cell weights + bias row
    # con1 weights in fp8 (x8 scale folded; /8 refolded into f1A):
    # DoubleRow pairs for H rows 0..255 of each half, tails (44 rows)
    # replicated at partitions 0-43 and 64-107 for even/odd-k con tiles.
    wpDc = din("wpDc", [128, 640], FP8)          # con half, DR [p, j, o];
    wpDl = din("wpDl", [128, 640], FP8)          # o padded 300->320 (the DR
    wpDc0 = din("wpDc0", [128, 640], FP8)        # ldweights AP needs the Ko
    wpDl0 = din("wpDl0", [128, 640], FP8)        # step to be 16B-aligned)
    wpTc = din("wpTc", [108, 300], FP8)
    wpTl = din("wpTl", [108, 300], FP8)
    wpTc0 = din("wpTc0", [108, 300], FP8)
    wpTl0 = din("wpTl0", [108, 300], FP8)
    bpn = din("bpn", [128, 3], F32)              # 8*bp per oc chunk
    bp0 = din("bp0", [128, 3], F32)              # tau0: 0 on core0, 8*bp else
    reluL = din("reluL", [128, 2], F32)          # relu floor: col0 tau0
                                                 # (-1e30 on core0), col1 0
    f1A = din("f1A", [300, 201], BF16)           # [v1 | F1top]
    f1B = din("f1B", [301, 201], BF16)           # [0 | F1bot] + b1 ones-row
    f2full = din("f2full", [128, 200], F32)
    # --- outputs / scratch ---
    preds = nc.dram_tensor("preds", [NRT, 128], F32, kind="ExternalOutput").ap()
    smflat = [nc.dram_tensor(f"smflat{t}", [B * NK], F32, kind="Internal").ap()
              for t in range(TLOC)]
    # write view: [p, i, k] -> flat k*512 + i*128 + p  (one DMA per tau)
    smW = [s.rearrange("(k i p) -> p i k", k=NK, i=4, p=128) for s in smflat]
    # read view: [p, i, j] -> flat (i*128+p)*10 + j  (one DMA per tau)
    smR = [s.rearrange("(i p j) -> p i j", i=4, p=128, j=NK) for s in smflat]
    predsW = preds.rearrange("(t i) p -> t p i", t=TLOC, i=4)

    with tile.TileContext(nc) as tc:
        with ExitStack() as ctx:
            wpool = ctx.enter_context(tc.tile_pool(name="wpool", bufs=1))
            conp = ctx.enter_context(tc.tile_pool(name="conp", bufs=3))
            con1p = ctx.enter_context(tc.tile_pool(name="con1p", bufs=2))
            htarp = ctx.enter_context(tc.tile_pool(name="htarp", bufs=1))
            scrp = ctx.enter_context(tc.tile_pool(name="scrp", bufs=2))
            rowp = ctx.enter_context(tc.tile_pool(name="rowp", bufs=1))
            smallp = ctx.enter_context(tc.tile_pool(name="smallp", bufs=2))
            xkp = ctx.enter_context(tc.tile_pool(name="xkp", bufs=8))
            psA = ctx.enter_context(tc.tile_pool(name="psA", bufs=1, space="PSUM"))
            psB = ctx.enter_context(tc.tile_pool(name="psB", bufs=1, space="PSUM"))
            psC = ctx.enter_context(tc.tile_pool(name="psC", bufs=1, space="PSUM"))

            # ---------------- phase W: load weights/constants ----------------
            def wtile(name, shape, dt, src):
                t = wpool.tile(shape, dt, tag=name)
                nc.sync.dma_start(t[:], src)
                return t

            wt_sb = wtile("wtT", [5, 900], BF16, wtT[:])
            ex_sb = wtile("exT", [5, R], BF16, exT[:])
            # neighbor-pair weight tiles: k0 rows 0-11, k1 rows 32-43 so the
            # two cells matmuls run in separate PE row groups concurrently
            wihP_sb = []
            for p in range(NK // 2):
                t = wpool.tile([44, 900], BF16, tag=f"wihP{p}")
                nc.sync.dma_start(t[0:12, :], wihT[2 * p])
                nc.sync.dma_start(t[32:44, :], wihT[2 * p + 1])
                wihP_sb.append(t)
            # con1 fp8 weight tiles (DR pairs + replicated tails)
            wpDc_sb = wtile("wpDc", [128, 2, 320], FP8,
                            wpDc.rearrange("p (j o) -> p j o", j=2))
            wpDl_sb = wtile("wpDl", [128, 2, 320], FP8,
                            wpDl.rearrange("p (j o) -> p j o", j=2))
            wpDc0_sb = wtile("wpDc0", [128, 2, 320], FP8,
                             wpDc0.rearrange("p (j o) -> p j o", j=2))
            wpDl0_sb = wtile("wpDl0", [128, 2, 320], FP8,
                             wpDl0.rearrange("p (j o) -> p j o", j=2))
            wpTc_sb = wtile("wpTc", [108, 300], FP8, wpTc[:])
            wpTl_sb = wtile("wpTl", [108, 300], FP8, wpTl[:])
            wpTc0_sb = wtile("wpTc0", [108, 300], FP8, wpTc0[:])
            wpTl0_sb = wtile("wpTl0", [108, 300], FP8, wpTl0[:])
            HSZ2 = [128, 128, 45]
            f1A_sb = [wtile(f"f1A{c}", [HC[c][1], 201], BF16,
                            f1A[HC[c][0]:HC[c][0] + HC[c][1], :]) for c in range(3)]
            f1B_sb = [wtile(f"f1B{c}", [HSZ2[c], 201], BF16,
                            f1B[HC[c][0]:HC[c][0] + HSZ2[c], :]) for c in range(3)]
            f2_sb = wtile("f2full", [128, 200], F32, f2full[:])
            # xrow (wext+fusc, host-folded) preloaded for all 4 taus
            xrt = [wtile(f"xrt{t}", [128, 4, 11], F32,
                         xrow[t * 512:(t + 1) * 512, :]
                         .rearrange("(i p) j -> p i j", i=4, p=128))
                   for t in range(TLOC)]
            bpn_sb = wtile("bpn", [128, 3], F32, bpn[:])
            bp0_sb = wtile("bp0", [128, 3], F32, bp0[:])
            rl_sb = wtile("reluL", [128, 2], F32, reluL[:])

            GOFF = {"i": 0, "g": 300, "o": 600}
            last_act = [None]
            # static PSUM tiles: pool.tile() per group costs ~1us of
            # TileRelease semaphore latency on the PE; fixed tiles rely on
            # direct producer/consumer deps only.
            psA_t = [psA.tile([128, 1024], F32, name=f"psAs{i}", tag=f"psAs{i}")
                     for i in range(2)]
            psB_t = [psB.tile([128, 512], F32, name=f"psBs{i}", tag=f"psBs{i}")
                     for i in range(2)]
            psC_t = [psC.tile([128, 512], F32, name=f"psCs{i}", tag=f"psCs{i}")
                     for i in range(2)]
            rrA = [0]
            rrB = [0]
            rrC = [0]

            def nextps(tiles, rr):
                t = tiles[rr[0] % len(tiles)]
                rr[0] += 1
                return t


            # LSTM-cell gate pipeline: 3 matmul-evictions + product chain.
            # mm_fn(g, ps) issues the gate matmuls into psum slice ps;
            # out_fn(j) gives the [rows, 512] dest AP for sub-chunk j.
            def gate_chain2(rows, W, nsub, mm_fn, out_fn, filler=None):
                tiles = {}
                for g, func in (("i", AF.Sigmoid), ("g", AF.Tanh), ("o", AF.Sigmoid)):
                    if g == "o" and filler is not None:
                        filler()
                    ps = nextps(psA_t, rrA)[0:rows, 0:W]
                    mm_fn(g, ps)
                    t = scrp.tile([rows, W], BF16, name=f"sc_{g}", tag=f"sc_{g}")
                    nc.scalar.activation(t[:], ps[:], func)
                    tiles[g] = t
                m1 = scrp.tile([rows, W], BF16, name="sc_m1", tag="sc_m1")
                nc.gpsimd.tensor_mul(m1[:], tiles["i"][:], tiles["g"][:])
                m2 = scrp.tile([rows, W], BF16, name="sc_m2", tag="sc_m2")
                last_act[0] = nc.scalar.activation(m2[:], m1[:], AF.Tanh).ins
                for j in range(nsub):
                    cols = slice(j * 512, (j + 1) * 512)
                    # fp8-dest muls stay on GPSIMD: a DVE fp8 write drops to
                    # 1x mode (~800ns/op, measured) and overloads the DVE.
                    nc.gpsimd.tensor_mul(out_fn(j), tiles["o"][:, cols],
                                         m2[:, cols])

            def gate_chain(c, W, nsub, lhsT_fn, rhs_fn, out_fn, filler=None):
                coff, csz = HC[c]

                def mm_fn(g, ps):
                    for j in range(nsub):
                        nc.tensor.matmul(ps[:, j * 512:(j + 1) * 512],
                                         lhsT_fn(GOFF[g] + coff, csz),
                                         rhs_fn(j), start=True, stop=True)

                gate_chain2(csz, W, nsub, mm_fn, out_fn, filler)

            # HAM warmup: dense K=128 MMs (small-K matmuls don't register as
            # PE-busy to the clock-gate activity monitor) flip the PE clock
            # 1.2 -> 2.4 GHz while input DMAs stream in; psB_t[0] is
            # overwritten garbage, first real use is much later.
            for _wu in range(64):
                nc.tensor.matmul(psB_t[0][0:128, 0:201], f1A_sb[0][:, 0:128],
                                 f1B_sb[0][:], start=True, stop=True)

            # ---------------- phase H: target cell (htar) ----------------
            HSZ = [128, 128, 45]   # c2 carries a ones row for the b1 bias
            htar = [htarp.tile([HSZ[c], R], BF16, name=f"htar{c}", tag=f"htar{c}") for c in range(3)]
            nc.sync.dma_start(htar[2][44:45, :], exT[4:5, :])
            for ccp in ((0, 1), (2, 3)):
                for c in range(3):
                    gate_chain(
                        c, 512 * len(ccp), len(ccp),
                        lambda off, sz: wt_sb[:, off:off + sz],
                        lambda j: ex_sb[:, (ccp[0] + j) * 512:(ccp[0] + j + 1) * 512],
                        lambda j, c=c, ccp=ccp: htar[c][0:HC[c][1],
                                                        (ccp[0] + j) * 512:
                                                        (ccp[0] + j + 1) * 512])

            conD_tiles = {}  # (taup, k) -> [128, 2, 512] fp8 (h rows 0..255)
            conT_tiles = {}  # (taup, kp) -> [108, 512] fp8 (2 k's tail rows)

            def emit_cells_pair(pi, kp, fillers=None):
                # neighbors (2kp, 2kp+1) computed concurrently: k0 weights/x
                # at partitions 0-11, k1 at 32-43 (separate PE row groups),
                # psum column halves per taup; their 44-row H-tails share one
                # chain at partitions 0-43 / 64-107. Fillers fire BETWEEN
                # chains as contiguous K>=128 fragments: a dozen back-to-back
                # fp8-DR matmuls register as PE-busy and lift the clock gate.
                tp = TPAIRS[pi]
                W = 512 * len(tp)
                k0, k1 = 2 * kp, 2 * kp + 1
                wP = wihP_sb[kp]
                xks = []
                for taup in tp:
                    xk1 = xkp.tile([44, 512], BF16, name="xk", tag="xk")
                    nc.sync.dma_start(
                        xk1[0:12, :], xT[k0, :, taup * 512:(taup + 1) * 512])
                    nc.sync.dma_start(
                        xk1[32:44, :], xT[k1, :, taup * 512:(taup + 1) * 512])
                    xks.append(xk1)
                for k in (k0, k1):
                    for taup in tp:
                        conD_tiles[(taup, k)] = conp.tile(
                            [128, 2, 512], FP8, name=f"conD{k}", tag=f"conD{k}")
                for taup in tp:
                    conT_tiles[(taup, kp)] = conp.tile(
                        [108, 512], FP8, name=f"conT{kp}", tag=f"conT{kp}")
                fl = fillers or [None] * 2
                chains = [(c, j) for c in (0, 1) for j in range(len(tp))]
                for idx, (c, j) in enumerate(chains):
                    coff = HC[c][0]

                    def mm_fn(g, ps, j=j, coff=coff):
                        o0 = GOFF[g] + coff
                        nc.tensor.matmul(ps[:, 0:512], wP[0:12, o0:o0 + 128],
                                         xks[j][0:12, :], start=True, stop=True)
                        nc.tensor.matmul(ps[:, 512:1024],
                                         wP[32:44, o0:o0 + 128],
                                         xks[j][32:44, :], start=True,
                                         stop=True, tile_position=(32, 0))

                    gate_chain2(
                        128, 1024, 2, mm_fn,
                        lambda jj, tp=tp, j=j, c=c:
                            conD_tiles[(tp[j], k0 if jj == 0 else k1)][:, c, :])
                    if idx == 1 and fl[0] is not None:
                        fl[0]()
                if fl[1] is not None:
                    fl[1]()

                def mm_tail(g, ps):
                    for j in range(len(tp)):
                        o0 = GOFF[g] + 256
                        nc.tensor.matmul(
                            ps[0:44, j * 512:(j + 1) * 512],
                            wP[0:12, o0:o0 + 44], xks[j][0:12, :],
                            start=True, stop=True, tile_position=(0, 0))
                        nc.tensor.matmul(
                            ps[64:108, j * 512:(j + 1) * 512],
                            wP[32:44, o0:o0 + 44], xks[j][32:44, :],
                            start=True, stop=True, tile_position=(32, 64))

                gate_chain2(108, W, len(tp), mm_tail,
                            lambda j, tp=tp, kp=kp: conT_tiles[(tp[j], kp)][:])

            srow = [rowp.tile([128, NK], F32, name=f"srow{rt}", tag=f"srow{rt}") for rt in range(NRT)]
            wdyn = [rowp.tile([128, 4 * NK], F32, name=f"wdyn{t}", tag=f"wdyn{t}") for t in range(TLOC)]

            con1_tiles = {}  # (tau, k, oc) -> tile

            def emit_con1_group(tau, k, oc):
                # con1(tau,k,oc) = relu(wp8 @ [con_t; con_tm1] + 8bp), fp8:
                # 2 DoubleRow MMs (K=256 each) + 2 tail MMs (K=44). The relu
                # floor comes from per-core data (core0 tau0: -1e30 =
                # passthrough of the identity-weight con0 path).
                ooff, osz = HC[oc]
                ps = nextps(psB_t, rrB)[0:osz, 0:512]
                if tau == 0:
                    wc, wl, tc_, tl_ = wpDc0_sb, wpDl0_sb, wpTc0_sb, wpTl0_sb
                    bsel, lcol = bp0_sb, 0
                else:
                    wc, wl, tc_, tl_ = wpDc_sb, wpDl_sb, wpTc_sb, wpTl_sb
                    bsel, lcol = bpn_sb, 1
                kp, base = k // 2, (k % 2) * 64
                nc.tensor.matmul(ps[:], wc[:, :, ooff:ooff + osz],
                                 conD_tiles[(tau + 1, k)][:],
                                 start=True, stop=False, perf_mode=DR)
                nc.tensor.matmul(ps[:], wl[:, :, ooff:ooff + osz],
                                 conD_tiles[(tau, k)][:],
                                 start=False, stop=False, perf_mode=DR)
                nc.tensor.matmul(ps[:], tc_[base:base + 44, ooff:ooff + osz],
                                 conT_tiles[(tau + 1, kp)][base:base + 44, :],
                                 start=False, stop=False)
                nc.tensor.matmul(ps[:], tl_[base:base + 44, ooff:ooff + osz],
                                 conT_tiles[(tau, kp)][base:base + 44, :],
                                 start=False, stop=True)
                ct = con1p.tile([osz, 512], BF16, name=f"con1_{k}_{oc}",
                                tag=f"con1_{k}_{oc}")
                nc.vector.scalar_tensor_tensor(
                    ct[:], ps[:], bsel[0:osz, oc:oc + 1],
                    rl_sb[0:osz, lcol:lcol + 1].broadcast_to([osz, 512]),
                    ALU.add, ALU.max)
                con1_tiles[(tau, k, oc)] = ct

            def emit_con1_k(tau, k):
                for oc in range(3):
                    emit_con1_group(tau, k, oc)

            def emit_fc1(tau):
                # fc1: Y = htar@F1bot (+b1 ones-row) once per row-tile, then
                # per-k psum = con1-part; col 0 collects s_k = v1 . con1_k.
                # Two neighbors share each psum tile.
                for rt4 in range(4):
                    rt = tau * 4 + rt4
                    cols = (rt * 128, rt * 128 + 128)
                    psY = nextps(psC_t, rrC)[:, 0:201]
                    for c in range(3):
                        nc.tensor.matmul(psY[:], htar[c][:, cols[0]:cols[1]],
                                         f1B_sb[c][:], start=(c == 0), stop=(c == 2))
                    ysb = smallp.tile([128, 201], F32, name="ysb", tag="ysb")
                    nc.vector.tensor_copy(ysb[:], psY[:])
                    ysb2 = ysb[:].unsqueeze(1).broadcast_to([128, 2, 201])
                    for k0 in range(0, NK, 2):
                        psF = nextps(psC_t, rrC)[:, 0:402]
                        for kk in (0, 1):
                            sl = slice(kk * 201, kk * 201 + 201)
                            for oc in range(3):
                                nc.tensor.matmul(
                                    psF[:, sl],
                                    con1_tiles[(tau, k0 + kk, oc)][:, rt4 * 128:rt4 * 128 + 128],
                                    f1A_sb[oc][:], start=(oc == 0), stop=(oc == 2))
                        f1 = smallp.tile([128, 402], F32, name="f1", tag="f1")
                        nc.vector.tensor_tensor(out=f1[:].rearrange("p (u q) -> p u q", u=2),
                                                in0=psF[:].rearrange("p (u q) -> p u q", u=2),
                                                in1=ysb2, op=ALU.add)
                        for kk in (0, 1):
                            nc.vector.tensor_copy(srow[rt][:, k0 + kk:k0 + kk + 1],
                                                  f1[:, kk * 201:kk * 201 + 1])
                            stsc = smallp.tile([128, 200], F32, name="stsc", tag="stsc")
                            nc.vector.scalar_tensor_tensor(
                                stsc[:], f1[:, kk * 201 + 1:kk * 201 + 201], 0.0,
                                f2_sb[:], ALU.max, ALU.mult,
                                accum_out=wdyn[tau][:, rt4 * NK + k0 + kk:
                                                    rt4 * NK + k0 + kk + 1])

            def emit_XS_head(tau):
                # wfin = relu(wdyn + wext) (wext host-folded incl b2 and the
                # fl8/angle terms), then softmax numerator/denominator and
                # the scramble write; all [128, 40] = 4 row-tiles at once.
                wextAP = xrt[tau][:, :, 0:10]
                wsum = smallp.tile([128, 4 * NK], F32, name="wsum", tag="wsum")
                nc.vector.scalar_tensor_tensor(
                    wsum[:].rearrange("p (i j) -> p i j", i=4),
                    wdyn[tau][:].rearrange("p (i j) -> p i j", i=4),
                    0.0, wextAP, ALU.add, ALU.add)
                wfin = smallp.tile([128, 4 * NK], F32, name="wfin", tag="wfin")
                nc.vector.tensor_scalar(wfin[:], wsum[:], 0.0, None, ALU.max)
                e = smallp.tile([128, 4 * NK], F32, name="e", tag="e")
                nc.scalar.activation(e[:], wfin[:], AF.Exp)
                z = smallp.tile([128, 4], F32, name="z", tag="z")
                nc.vector.tensor_reduce(z[:],
                                        e[:].rearrange("p (i j) -> p i j", i=4),
                                        mybir.AxisListType.X, ALU.add)
                rz = smallp.tile([128, 4], F32, name="rz", tag="rz")
                nc.vector.reciprocal(rz[:], z[:])
                smtau = smallp.tile([128, 4, NK], F32, name="smtau", tag="smtau")
                nc.vector.tensor_tensor(
                    out=smtau[:], in0=e[:].rearrange("p (i j) -> p i j", i=4),
                    in1=rz[:].unsqueeze(2).broadcast_to([128, 4, NK]),
                    op=ALU.mult)
                for i4 in range(4):
                    nc.sync.dma_start(smW[tau][:, i4], smtau[:, i4, :])

            # ---------------- main pipeline ----------------

            # cells chains interleaved with con1 groups at lag 2, so the PE
            # never head-of-line blocks on a chain's ACT/GP/DVE pipeline.
            def stretch(pi, tau):
                # cells pairs 0..4; the previous pair's con1 groups are woven
                # in as two 3-group (12-MM) fragments per pair.
                for s in range(6):
                    fl = None
                    if s >= 1:
                        ka, kb = 2 * (s - 1), 2 * (s - 1) + 1
                        fl = [
                            (lambda ka=ka: emit_con1_k(tau, ka)),
                            (lambda kb=kb: emit_con1_k(tau, kb)),
                        ]
                    if s < 5:
                        emit_cells_pair(pi, s, fillers=fl)
                    elif fl:
                        for f in fl:
                            f()

            def emit_S_tail(tau):
                # scramble read-back + weighted-sum + output; emitted a full
                # phase after emit_XS_head so the DRAM round trip is hidden.
                wa3t = smallp.tile([128, 4 * NK], F32, name="wa3t", tag="wa3t")
                for i4 in range(4):
                    nc.sync.dma_start(wa3t[:, i4 * NK:(i4 + 1) * NK],
                                      smR[tau][:, i4])
                prt = smallp.tile([128, 4], F32, name="prt", tag="prt")
                for i4 in range(4):
                    rt = tau * 4 + i4
                    csc = smallp.tile([128, NK], F32, name="csc", tag="csc")
                    cat = smallp.tile([128, 1], F32, name="cat", tag="cat")
                    nc.vector.scalar_tensor_tensor(
                        csc[:], srow[rt][:], 0.0,
                        wa3t[:, i4 * NK:(i4 + 1) * NK],
                        ALU.add, ALU.mult, accum_out=cat[:])
                    nc.vector.scalar_tensor_tensor(
                        prt[:, i4:i4 + 1], cat[:], 0.0,
                        xrt[tau][:, i4, 10:11], ALU.add, ALU.add)
                nc.sync.dma_start(predsW[tau], prt[:])

            # XS_head right after its fc1 (cheap: ~6 ops); S_tail a full
            # phase later so the smW->smR DRAM round trip never stalls any
            # engine queue (>3.4us PE-idle would re-throttle the clock gate).
            stretch(0, 0)
            emit_fc1(0)
            emit_XS_head(0)
            stretch(1, 1)
            emit_S_tail(0)
            emit_fc1(1)
            emit_XS_head(1)
            for k in range(NK):
                emit_con1_k(2, k)
            emit_S_tail(1)
            emit_fc1(2)
            emit_XS_head(2)
            stretch(2, 3)
            emit_S_tail(2)
            emit_fc1(3)
            emit_XS_head(3)
            emit_S_tail(3)

    nc.compile()
    nc.m = get_hw_module(nc.m)
    return nc


def _get_nc():
    if "nc" not in _BUILD_CACHE:
        _BUILD_CACHE["nc"] = _build()
    return _BUILD_CACHE["nc"]


def _softmax(x):
    e = np.exp(x - x.max())
    return e / e.sum()


def prepare_inputs(local_inputs, labels, extras, DisM, AngleM,
                   Wih, b_ih, b_hh, Wt, bt_ih, bt_hh,
                   wp, bp, F1, b1, F2, b2, ff, bff,
                   fuse1, biasf, Wout, biasout, a):
    """Host-side sharding + layout prep. Returns (in_maps, scalars)."""
    f = np.asarray
    local_inputs, labels, extras = f(local_inputs), f(labels), f(extras)
    aa = float(f(a)[0])
    wA = _softmax(f(DisM).astype(np.float64)).astype(np.float32)

    kept = np.r_[0:300, 600:900, 900:1200]      # gates i, g, o (f is dead)
    b_cells = (f(b_ih) + f(b_hh))[:, kept]      # [10, 900]
    bt = (f(bt_ih) + f(bt_hh))[kept]            # [900]

    # wihT[k]: [12, 900] = [Wih[k,kept,:].T ; bias row]
    wihT = np.zeros((NK, 12, 900), np.float32)
    for k in range(NK):
        wihT[k, :11] = f(Wih)[k][kept, :].T
        wihT[k, 11] = b_cells[k]
    wtT = np.zeros((5, 900), np.float32)
    wtT[:4] = f(Wt)[kept, :].T
    wtT[4] = bt

    # con1 weights x8 in fp8 (the /8 is refolded into f1A below)
    wp8 = (8.0 * f(wp).T).astype(np.float32)    # [600, 300]
    I8 = np.zeros((600, 300), np.float32)
    I8[:300] = 8.0 * np.eye(300, dtype=np.float32)

    def pack_con1(w):
        Dc = np.zeros((128, 2, 320), np.float32)
        Dc[:, :, :300] = w[0:256].reshape(2, 128, 300).transpose(1, 0, 2)
        Dc = Dc.reshape(128, 640)
        Dl = np.zeros((128, 2, 320), np.float32)
        Dl[:, :, :300] = w[300:556].reshape(2, 128, 300).transpose(1, 0, 2)
        Dl = Dl.reshape(128, 640)
        Tc = np.zeros((108, 300), np.float32)
        Tc[0:44] = w[256:300]
        Tc[64:108] = w[256:300]
        Tl = np.zeros((108, 300), np.float32)
        Tl[0:44] = w[556:600]
        Tl[64:108] = w[556:600]
        return (Dc.astype(NPF8), Dl.astype(NPF8),
                Tc.astype(NPF8), Tl.astype(NPF8))

    wpN = pack_con1(wp8)
    wp0 = pack_con1(I8)

    v1 = aa * (f(fuse1) @ f(Wout))[:, 0]        # [300]
    f1A = np.zeros((300, 201), np.float32)
    f1A[:, 0] = v1 / 8.0
    f1A[:, 1:] = f(F1)[:300] / 8.0
    f1B = np.zeros((301, 201), np.float32)
    f1B[:300, 1:] = f(F1)[300:]
    f1B[300, 1:] = f(b1)          # ones-row bias (htar c2 row 44)
    f2full = np.broadcast_to(f(F2)[:200, 0][None, :], (128, 200)).copy()

    ffW = (f(Wout)[:, 0] @ f(ff))               # [17]
    W3 = (1.0 - aa) * np.outer(ffW, wA)         # [17, 10]

    f2_8 = float(f(F2)[200, 0])
    f2a = float(f(F2)[201, 0]) / 360.0
    b2s = float(f(b2)[0])
    c0 = ((1.0 - aa) * float(f(Wout)[:, 0] @ f(bff)[:, 0])
          + aa * float(f(biasf) @ f(Wout)[:, 0])
          + float(f(biasout)[0]))

    bpcol = np.zeros((128, 3), np.float32)
    for oc, (ooff, osz) in enumerate(HC):
        bpcol[:osz, oc] = 8.0 * f(bp)[ooff:ooff + osz, 0]

    in_maps = []
    for cix in range(NCORES):
        t0 = cix * TLOC
        # x block for cells: t0-1 .. t0+3 (zeros for t=-1 on core 0)
        xblk = np.zeros((5, B, 28, NK), np.float32)
        lo = t0 - 1
        for jj in range(5):
            t = lo + jj
            if 0 <= t < T:
                xblk[jj] = local_inputs[:, t]
        xT = np.zeros((NK, 12, 5 * B), np.float32)
        # xT[k, i, taup*512+b] = xblk[taup, b, i, k]
        xT[:, :11, :] = xblk[:, :, :11, :].transpose(3, 2, 0, 1).reshape(NK, 11, 5 * B)
        xT[:, 11, :] = 1.0

        xloc = local_inputs[:, t0:t0 + TLOC]            # [B, 4, 28, 10]
        # host-fold the wdyn extras and the fusiondis term:
        #   wext = F2[200]*fl8 + (F2[201]/360)*|fl10 - Angle| + b2
        #   fusc = xfus . W3 + c0
        fl8 = xloc[:, :, 8, :].transpose(1, 0, 2).reshape(R, NK)
        fl10 = xloc[:, :, 10, :].transpose(1, 0, 2).reshape(R, NK)
        xfus = xloc[:, :, 11:, :].transpose(1, 0, 2, 3).reshape(R, 170)
        xrow = np.empty((R, 11), np.float32)
        xrow[:, 0:10] = (f2_8 * fl8
                         + f2a * np.abs(fl10 - f(AngleM)[None, :]) + b2s)
        xrow[:, 10] = xfus @ W3.reshape(-1) + c0

        exT = np.ones((5, R), np.float32)
        exT[:4] = f(extras)[:, t0:t0 + TLOC, :4, 0].transpose(2, 1, 0).reshape(4, R)

        core0 = cix == 0
        wpz = wp0 if core0 else wpN
        rl = np.zeros((128, 2), np.float32)
        if core0:
            rl[:, 0] = -1e30
        in_maps.append({
            "xT": xT.astype(NPBF),
            "xrow": xrow,
            "exT": exT.astype(NPBF),
            "wihT": wihT.astype(NPBF),
            "wtT": wtT.astype(NPBF),
            "wpDc": wpN[0], "wpDl": wpN[1], "wpTc": wpN[2], "wpTl": wpN[3],
            "wpDc0": wpz[0], "wpDl0": wpz[1], "wpTc0": wpz[2], "wpTl0": wpz[3],
            "bpn": bpcol,
            "bp0": (np.zeros((128, 3), np.float32) if core0 else bpcol),
            "reluL": rl,
            "f1A": f1A.astype(NPBF),
            "f1B": f1B.astype(NPBF),
            "f2full": f2full,
        })
    return in_maps


def kernel(local_inputs, labels, extras, DisM, AngleM,
           Wih, b_ih, b_hh, Wt, bt_ih, bt_hh,
           wp, bp, F1, b1, F2, b2, ff, bff,
           fuse1, biasf, Wout, biasout, a, _trace=False, _tmpdir=None):
    in_maps = prepare_inputs(
        local_inputs, labels, extras, DisM, AngleM,
        Wih, b_ih, b_hh, Wt, bt_ih, bt_hh, wp, bp, F1, b1, F2, b2,
        ff, bff, fuse1, biasf, Wout, biasout, a)
    nc = _get_nc()
    res = bass_utils.run_bass_kernel_spmd(
        nc, in_maps, core_ids=list(range(NCORES)), trace=_trace, tmpdir=_tmpdir)

    preds = np.empty((T, B, 1), np.float32)
    for cix in range(NCORES):
        out = res.results[cix]["preds"].reshape(TLOC, B)
        preds[cix * TLOC:(cix + 1) * TLOC, :, 0] = out

    labels_r = np.ascontiguousarray(
        np.transpose(np.asarray(labels), (1, 0, 2, 3)).reshape(T, B, 1))
    kernel._last_result = res
    return preds, labels_r



# revision 54
# speedup vs baseline: 1.0233x; 1.0233x over previous
"""Trainium2 Bass kernel for nn_GCLSTM (B=512, T=32, H=300, 10 neighbors).

Sharding: T is split across 8 cores (4 timesteps each). The reference's
scan carry (`con`) is a pure function of the per-step input, so every
timestep's cells() output can be computed independently; each core only
additionally computes cells() for its t0-1 block to get `lastcon`.
The flat-reshape softmax scramble mixes the whole batch at fixed t, so
keeping B intact per core makes it core-local.

The t=0 step (core 0 only) uses con0 directly instead of
relu(wp@[con;lastcon]+bp); this is folded into the uniform SPMD program
by giving core 0 identity/zero weights for its tau=0 con1 matmul and a
per-core relu floor (-1e30 on core 0 -> passthrough) in the eviction.

Perf notes (measured on this trn2): con and wp are held in fp8e4m3 with an
x8 scale (DoubleRow matmuls, K=256/instr); neighbor pairs run in separate
PE row groups concurrently; the two neighbors' 44-row H-tails share one
psum/ACT/mul chain; con1 groups are emitted as contiguous 12-MM fragments
(long enough to register as PE-busy for the HAM clock gate); fp8-dest muls
stay on GPSIMD (DVE fp8 writes drop to 1x); the wdyn extras/fusiondis
terms are host-folded into xrow; emit_S is split so its DRAM scramble
round trip never heads any engine queue.
"""
import os
import sys

for _p in ("/opt/trn_rl_repo", "/root/.axon_site", "/root/.axon_site/_ro/trn_rl_repo",
           "/root/.axon_site/_ro/pypackages"):
    if os.path.isdir(_p) and _p not in sys.path:
        sys.path.append(_p)

import numpy as np
import ml_dtypes
from contextlib import ExitStack

import concourse.bass as bass
import concourse.tile as tile
from concourse.tile_rust import add_dep_helper
from concourse import bacc, mybir
from concourse import bass_utils
from concourse.bass_interp import get_hw_module

BF16 = mybir.dt.bfloat16
F32 = mybir.dt.float32
FP8 = mybir.dt.float8e4
NPBF = ml_dtypes.bfloat16
NPF8 = ml_dtypes.float8_e4m3
AF = mybir.ActivationFunctionType
ALU = mybir.AluOpType
DR = mybir.MatmulPerfMode.DoubleRow

B, T, H = 512, 32, 300
NCORES = 8
TLOC = T // NCORES            # 4 timesteps per core
R = TLOC * B                  # 2048 rows per core (tau*512 + b)
NK = 10                       # neighbors
HC = [(0, 128), (128, 128), (256, 44)]   # H=300 chunking (offset, size)
NRT = R // 128                # 16 row-tiles of 128
# tau' cell blocks: 5 per core (t0-1 .. t0+3), processed in pairs
TPAIRS = [(0, 1), (2, 3), (4,)]


_BUILD_CACHE = {}


def _build():
    nc = bacc.Bacc("TRN2", target_bir_lowering=False, debug=False,
                   enable_asserts=False, num_devices=NCORES)

    def din(name, shape, dt):
        return nc.dram_tensor(name, shape, dt, kind="ExternalInput").ap()

    # --- per-core data inputs ---
    xT = din("xT", [NK, 12, 5 * B], BF16)        # [k, i(11)+ones, tau'*512+b]
    xrow = din("xrow", [R, 11], F32)             # wext(10) fusc(1), host-folded
    exT = din("exT", [5, R], BF16)               # extras features + ones row
    # --- weights (replicated; wpT0/bp0/alpha differ on core 0) ---
    wihT = din("wihT", [NK, 12, 900], BF16)      # per-k lstm weights + bias row
    wtT = din("wtT", [5, 900], BF16)             # target-cell weights + bias row
    # con1 weights in fp8 (x8 scale folded; /8 refolded into f1A):
    # DoubleRow pairs for H rows 0..255 of each half, tails (44 rows)
    # replicated at partitions 0-43 and 64-107 for even/odd-k con tiles.
    wpDc = din("wpDc", [128, 640], FP8)          # con half, DR [p, j, o];
    wpDl = din("wpDl", [128, 640], FP8)          # o padded 300->320 (the DR
    wpDc0 = din("wpDc0", [128, 640], FP8)        # ldweights AP needs the Ko
    wpDl0 = din("wpDl0", [128, 640], FP8)        # step to be 16B-aligned)
    wpTc = din("wpTc", [108, 300], FP8)
    wpTl = din("wpTl", [108, 300], FP8)
    wpTc0 = din("wpTc0", [108, 300], FP8)
    wpTl0 = din("wpTl0", [108, 300], FP8)
    bpn = din("bpn", [128, 3], F32)              # 8*bp per oc chunk
    bp0 = din("bp0", [128, 3], F32)              # tau0: 0 on core0, 8*bp else
    reluL = din("reluL", [128, 2], F32)          # relu floor: col0 tau0
                                                 # (-1e30 on core0), col1 0
    f1A = din("f1A", [300, 201], BF16)           # [v1 | F1top]
    f1B = din("f1B", [301, 201], BF16)           # [0 | F1bot] + b1 ones-row
    f2full = din("f2full", [128, 200], F32)
    # --- outputs / scratch ---
    preds = nc.dram_tensor("preds", [NRT, 128], F32, kind="ExternalOutput").ap()
    smflat = [nc.dram_tensor(f"smflat{t}", [B * NK], F32, kind="Internal").ap()
              for t in range(TLOC)]
    # write view: [p, i, k] -> flat k*512 + i*128 + p  (one DMA per tau)
    smW = [s.rearrange("(k i p) -> p i k", k=NK, i=4, p=128) for s in smflat]
    # read view: [p, i, j] -> flat (i*128+p)*10 + j  (one DMA per tau)
    smR = [s.rearrange("(i p j) -> p i j", i=4, p=128, j=NK) for s in smflat]
    predsW = preds.rearrange("(t i) p -> t p i", t=TLOC, i=4)

    with tile.TileContext(nc) as tc:
        with ExitStack() as ctx:
            wpool = ctx.enter_context(tc.tile_pool(name="wpool", bufs=1))
            conp = ctx.enter_context(tc.tile_pool(name="conp", bufs=3))
            con1p = ctx.enter_context(tc.tile_pool(name="con1p", bufs=2))
            htarp = ctx.enter_context(tc.tile_pool(name="htarp", bufs=1))
            scrp = ctx.enter_context(tc.tile_pool(name="scrp", bufs=2))
            rowp = ctx.enter_context(tc.tile_pool(name="rowp", bufs=1))
            smallp = ctx.enter_context(tc.tile_pool(name="smallp", bufs=2))
            xkp = ctx.enter_context(tc.tile_pool(name="xkp", bufs=8))
            psA = ctx.enter_context(tc.tile_pool(name="psA", bufs=1, space="PSUM"))
            psB = ctx.enter_context(tc.tile_pool(name="psB", bufs=1, space="PSUM"))
            psC = ctx.enter_context(tc.tile_pool(name="psC", bufs=1, space="PSUM"))

            # ---------------- phase W: load weights/constants ----------------
            def wtile(name, shape, dt, src):
                t = wpool.tile(shape, dt, tag=name)
                nc.sync.dma_start(t[:], src)
                return t

            wt_sb = wtile("wtT", [5, 900], BF16, wtT[:])
            ex_sb = wtile("exT", [5, R], BF16, exT[:])
            # neighbor-pair weight tiles: k0 rows 0-11, k1 rows 32-43 so the
            # two cells matmuls run in separate PE row groups concurrently
            wihP_sb = []
            for p in range(NK // 2):
                t = wpool.tile([44, 900], BF16, tag=f"wihP{p}")
                nc.sync.dma_start(t[0:12, :], wihT[2 * p])
                nc.sync.dma_start(t[32:44, :], wihT[2 * p + 1])
                wihP_sb.append(t)
            # con1 fp8 weight tiles (DR pairs + replicated tails)
            wpDc_sb = wtile("wpDc", [128, 2, 320], FP8,
                            wpDc.rearrange("p (j o) -> p j o", j=2))
            wpDl_sb = wtile("wpDl", [128, 2, 320], FP8,
                            wpDl.rearrange("p (j o) -> p j o", j=2))
            wpDc0_sb = wtile("wpDc0", [128, 2, 320], FP8,
                             wpDc0.rearrange("p (j o) -> p j o", j=2))
            wpDl0_sb = wtile("wpDl0", [128, 2, 320], FP8,
                             wpDl0.rearrange("p (j o) -> p j o", j=2))
            wpTc_sb = wtile("wpTc", [108, 300], FP8, wpTc[:])
            wpTl_sb = wtile("wpTl", [108, 300], FP8, wpTl[:])
            wpTc0_sb = wtile("wpTc0", [108, 300], FP8, wpTc0[:])
            wpTl0_sb = wtile("wpTl0", [108, 300], FP8, wpTl0[:])
            HSZ2 = [128, 128, 45]
            f1A_sb = [wtile(f"f1A{c}", [HC[c][1], 201], BF16,
                            f1A[HC[c][0]:HC[c][0] + HC[c][1], :]) for c in range(3)]
            f1B_sb = [wtile(f"f1B{c}", [HSZ2[c], 201], BF16,
                            f1B[HC[c][0]:HC[c][0] + HSZ2[c], :]) for c in range(3)]
            f2_sb = wtile("f2full", [128, 200], F32, f2full[:])
            # xrow (wext+fusc, host-folded) preloaded for all 4 taus
            xrt = [wtile(f"xrt{t}", [128, 4, 11], F32,
                         xrow[t * 512:(t + 1) * 512, :]
                         .rearrange("(i p) j -> p i j", i=4, p=128))
                   for t in range(TLOC)]
            bpn_sb = wtile("bpn", [128, 3], F32, bpn[:])
            bp0_sb = wtile("bp0", [128, 3], F32, bp0[:])
            rl_sb = wtile("reluL", [128, 2], F32, reluL[:])

            GOFF = {"i": 0, "g": 300, "o": 600}
            last_act = [None]
            # static PSUM tiles: pool.tile() per group costs ~1us of
            # TileRelease semaphore latency on the PE; fixed tiles rely on
            # direct producer/consumer deps only.
            psA_t = [psA.tile([128, 1024], F32, name=f"psAs{i}", tag=f"psAs{i}")
                     for i in range(2)]
            psB_t = [psB.tile([128, 512], F32, name=f"psBs{i}", tag=f"psBs{i}")
                     for i in range(2)]
            psC_t = [psC.tile([128, 512], F32, name=f"psCs{i}", tag=f"psCs{i}")
                     for i in range(2)]
            rrA = [0]
            rrB = [0]
            rrC = [0]

            def nextps(tiles, rr):
                t = tiles[rr[0] % len(tiles)]
                rr[0] += 1
                return t


            # LSTM-cell gate pipeline: 3 matmul-evictions + product chain.
            # mm_fn(g, ps) issues the gate matmuls into psum slice ps;
            # out_fn(j) gives the [rows, 512] dest AP for sub-chunk j.
            def gate_chain2(rows, W, nsub, mm_fn, out_fn, filler=None):
                tiles = {}
                for g, func in (("i", AF.Sigmoid), ("g", AF.Tanh), ("o", AF.Sigmoid)):
                    if g == "o" and filler is not None:
                        filler()
                    ps = nextps(psA_t, rrA)[0:rows, 0:W]
                    mm_fn(g, ps)
                    t = scrp.tile([rows, W], BF16, name=f"sc_{g}", tag=f"sc_{g}")
                    nc.scalar.activation(t[:], ps[:], func)
                    tiles[g] = t
                m1 = scrp.tile([rows, W], BF16, name="sc_m1", tag="sc_m1")
                nc.gpsimd.tensor_mul(m1[:], tiles["i"][:], tiles["g"][:])
                m2 = scrp.tile([rows, W], BF16, name="sc_m2", tag="sc_m2")
                last_act[0] = nc.scalar.activation(m2[:], m1[:], AF.Tanh).ins
                for j in range(nsub):
                    cols = slice(j * 512, (j + 1) * 512)
                    # fp8-dest muls stay on GPSIMD: a DVE fp8 write drops to
                    # 1x mode (~800ns/op, measured) and overloads the DVE.
                    nc.gpsimd.tensor_mul(out_fn(j), tiles["o"][:, cols],
                                         m2[:, cols])

            def gate_chain(c, W, nsub, lhsT_fn, rhs_fn, out_fn, filler=None):
                coff, csz = HC[c]

                def mm_fn(g, ps):
                    for j in range(nsub):
                        nc.tensor.matmul(ps[:, j * 512:(j + 1) * 512],
                                         lhsT_fn(GOFF[g] + coff, csz),
                                         rhs_fn(j), start=True, stop=True)

                gate_chain2(csz, W, nsub, mm_fn, out_fn, filler)

            # HAM warmup: dense K=128 MMs (small-K matmuls don't register as
            # PE-busy to the clock-gate activity monitor) flip the PE clock
            # 1.2 -> 2.4 GHz while input DMAs stream in; psB_t[0] is
            # overwritten garbage, first real use is much later.
            for _wu in range(64):
                nc.tensor.matmul(psB_t[0][0:128, 0:201], f1A_sb[0][:, 0:128],
                                 f1B_sb[0][:], start=True, stop=True)

            # ---------------- phase H: target cell (htar) ----------------
            HSZ = [128, 128, 45]   # c2 carries a ones row for the b1 bias
            htar = [htarp.tile([HSZ[c], R], BF16, name=f"htar{c}", tag=f"htar{c}") for c in range(3)]
            nc.sync.dma_start(htar[2][44:45, :], exT[4:5, :])
            for ccp in ((0, 1), (2, 3)):
                for c in range(3):
                    gate_chain(
                        c, 512 * len(ccp), len(ccp),
                        lambda off, sz: wt_sb[:, off:off + sz],
                        lambda j: ex_sb[:, (ccp[0] + j) * 512:(ccp[0] + j + 1) * 512],
                        lambda j, c=c, ccp=ccp: htar[c][0:HC[c][1],
                                                        (ccp[0] + j) * 512:
                                                        (ccp[0] + j + 1) * 512])

            conD_tiles = {}  # (taup, k) -> [128, 2, 512] fp8 (h rows 0..255)
            conT_tiles = {}  # (taup, kp) -> [108, 512] fp8 (2 k's tail rows)

            def emit_cells_pair(pi, kp, fillers=None):
                # neighbors (2kp, 2kp+1) computed concurrently: k0 weights/x
                # at partitions 0-11, k1 at 32-43 (separate PE row groups),
                # psum column halves per taup; their 44-row H-tails share one
                # chain at partitions 0-43 / 64-107. Fillers fire BETWEEN
                # chains as contiguous K>=128 fragments: a dozen back-to-back
                # fp8-DR matmuls register as PE-busy and lift the clock gate.
                tp = TPAIRS[pi]
                W = 512 * len(tp)
                k0, k1 = 2 * kp, 2 * kp + 1
                wP = wihP_sb[kp]
                xks = []
                for taup in tp:
                    xk1 = xkp.tile([44, 512], BF16, name="xk", tag="xk")
                    nc.sync.dma_start(
                        xk1[0:12, :], xT[k0, :, taup * 512:(taup + 1) * 512])
                    nc.sync.dma_start(
                        xk1[32:44, :], xT[k1, :, taup * 512:(taup + 1) * 512])
                    xks.append(xk1)
                for k in (k0, k1):
                    for taup in tp:
                        conD_tiles[(taup, k)] = conp.tile(
                            [128, 2, 512], FP8, name=f"conD{k}", tag=f"conD{k}")
                for taup in tp:
                    conT_tiles[(taup, kp)] = conp.tile(
                        [108, 512], FP8, name=f"conT{kp}", tag=f"conT{kp}")
                fl = fillers or [None] * 2
                chains = [(c, j) for c in (0, 1) for j in range(len(tp))]
                for idx, (c, j) in enumerate(chains):
                    coff = HC[c][0]

                    def mm_fn(g, ps, j=j, coff=coff):
                        o0 = GOFF[g] + coff
                        nc.tensor.matmul(ps[:, 0:512], wP[0:12, o0:o0 + 128],
                                         xks[j][0:12, :], start=True, stop=True)
                        nc.tensor.matmul(ps[:, 512:1024],
                                         wP[32:44, o0:o0 + 128],
                                         xks[j][32:44, :], start=True,
                                         stop=True, tile_position=(32, 0))

                    gate_chain2(
                        128, 1024, 2, mm_fn,
                        lambda jj, tp=tp, j=j, c=c:
                            conD_tiles[(tp[j], k0 if jj == 0 else k1)][:, c, :])
                    if idx == 1 and fl[0] is not None:
                        fl[0]()
                if fl[1] is not None:
                    fl[1]()

                def mm_tail(g, ps):
                    for j in range(len(tp)):
                        o0 = GOFF[g] + 256
                        nc.tensor.matmul(
                            ps[0:44, j * 512:(j + 1) * 512],
                            wP[0:12, o0:o0 + 44], xks[j][0:12, :],
                            start=True, stop=True, tile_position=(0, 0))
                        nc.tensor.matmul(
                            ps[64:108, j * 512:(j + 1) * 512],
                            wP[32:44, o0:o0 + 44], xks[j][32:44, :],
                            start=True, stop=True, tile_position=(32, 64))

                gate_chain2(108, W, len(tp), mm_tail,
                            lambda j, tp=tp, kp=kp: conT_tiles[(tp[j], kp)][:])

            srow = [rowp.tile([128, NK], F32, name=f"srow{rt}", tag=f"srow{rt}") for rt in range(NRT)]
            wdyn = [rowp.tile([128, 4 * NK], F32, name=f"wdyn{t}", tag=f"wdyn{t}") for t in range(TLOC)]

            con1_tiles = {}  # (tau, k, oc) -> tile

            def emit_con1_group(tau, k, oc):
                # con1(tau,k,oc) = relu(wp8 @ [con_t; con_tm1] + 8bp), fp8:
                # 2 DoubleRow MMs (K=256 each) + 2 tail MMs (K=44). The relu
                # floor comes from per-core data (core0 tau0: -1e30 =
                # passthrough of the identity-weight con0 path).
                ooff, osz = HC[oc]
                ps = nextps(psB_t, rrB)[0:osz, 0:512]
                if tau == 0:
                    wc, wl, tc_, tl_ = wpDc0_sb, wpDl0_sb, wpTc0_sb, wpTl0_sb
                    bsel, lcol = bp0_sb, 0
                else:
                    wc, wl, tc_, tl_ = wpDc_sb, wpDl_sb, wpTc_sb, wpTl_sb
                    bsel, lcol = bpn_sb, 1
                kp, base = k // 2, (k % 2) * 64
                nc.tensor.matmul(ps[:], wc[:, :, ooff:ooff + osz],
                                 conD_tiles[(tau + 1, k)][:],
                                 start=True, stop=False, perf_mode=DR)
                nc.tensor.matmul(ps[:], wl[:, :, ooff:ooff + osz],
                                 conD_tiles[(tau, k)][:],
                                 start=False, stop=False, perf_mode=DR)
                nc.tensor.matmul(ps[:], tc_[base:base + 44, ooff:ooff + osz],
                                 conT_tiles[(tau + 1, kp)][base:base + 44, :],
                                 start=False, stop=False)
                nc.tensor.matmul(ps[:], tl_[base:base + 44, ooff:ooff + osz],
                                 conT_tiles[(tau, kp)][base:base + 44, :],
                                 start=False, stop=True)
                ct = con1p.tile([osz, 512], BF16, name=f"con1_{k}_{oc}",
                                tag=f"con1_{k}_{oc}")
                nc.vector.scalar_tensor_tensor(
                    ct[:], ps[:], bsel[0:osz, oc:oc + 1],
                    rl_sb[0:osz, lcol:lcol + 1].broadcast_to([osz, 512]),
                    ALU.add, ALU.max)
                con1_tiles[(tau, k, oc)] = ct

            def emit_con1_k(tau, k):
                for oc in range(3):
                    emit_con1_group(tau, k, oc)

            def emit_fc1(tau):
                # fc1: Y = htar@F1bot (+b1 ones-row) once per row-tile, then
                # per-k psum = con1-part; col 0 collects s_k = v1 . con1_k.
                # Two neighbors share each psum tile.
                for rt4 in range(4):
                    rt = tau * 4 + rt4
                    cols = (rt * 128, rt * 128 + 128)
                    psY = nextps(psC_t, rrC)[:, 0:201]
                    for c in range(3):
                        nc.tensor.matmul(psY[:], htar[c][:, cols[0]:cols[1]],
                                         f1B_sb[c][:], start=(c == 0), stop=(c == 2))
                    ysb = smallp.tile([128, 201], F32, name="ysb", tag="ysb")
                    nc.vector.tensor_copy(ysb[:], psY[:])
                    ysb2 = ysb[:].unsqueeze(1).broadcast_to([128, 2, 201])
                    for k0 in range(0, NK, 2):
                        psF = nextps(psC_t, rrC)[:, 0:402]
                        for kk in (0, 1):
                            sl = slice(kk * 201, kk * 201 + 201)
                            for oc in range(3):
                                nc.tensor.matmul(
                                    psF[:, sl],
                                    con1_tiles[(tau, k0 + kk, oc)][:, rt4 * 128:rt4 * 128 + 128],
                                    f1A_sb[oc][:], start=(oc == 0), stop=(oc == 2))
                        f1 = smallp.tile([128, 402], F32, name="f1", tag="f1")
                        nc.vector.tensor_tensor(out=f1[:].rearrange("p (u q) -> p u q", u=2),
                                                in0=psF[:].rearrange("p (u q) -> p u q", u=2),
                                                in1=ysb2, op=ALU.add)
                        for kk in (0, 1):
                            nc.vector.tensor_copy(srow[rt][:, k0 + kk:k0 + kk + 1],
                                                  f1[:, kk * 201:kk * 201 + 1])
                            stsc = smallp.tile([128, 200], F32, name="stsc", tag="stsc")
                            nc.vector.scalar_tensor_tensor(
                                stsc[:], f1[:, kk * 201 + 1:kk * 201 + 201], 0.0,
                                f2_sb[:], ALU.max, ALU.mult,
                                accum_out=wdyn[tau][:, rt4 * NK + k0 + kk:
                                                    rt4 * NK + k0 + kk + 1])

            def emit_XS_head(tau):
                # wfin = relu(wdyn + wext) (wext host-folded incl b2 and the
                # fl8/angle terms), then softmax numerator/denominator and
                # the scramble write; all [128, 40] = 4 row-tiles at once.
                wextAP = xrt[tau][:, :, 0:10]
                wsum = smallp.tile([128, 4 * NK], F32, name="wsum", tag="wsum")
                nc.vector.scalar_tensor_tensor(
                    wsum[:].rearrange("p (i j) -> p i j", i=4),
                    wdyn[tau][:].rearrange("p (i j) -> p i j", i=4),
                    0.0, wextAP, ALU.add, ALU.add)
                wfin = smallp.tile([128, 4 * NK], F32, name="wfin", tag="wfin")
                nc.vector.tensor_scalar(wfin[:], wsum[:], 0.0, None, ALU.max)
                e = smallp.tile([128, 4 * NK], F32, name="e", tag="e")
                nc.scalar.activation(e[:], wfin[:], AF.Exp)
                z = smallp.tile([128, 4], F32, name="z", tag="z")
                nc.vector.tensor_reduce(z[:],
                                        e[:].rearrange("p (i j) -> p i j", i=4),
                                        mybir.AxisListType.X, ALU.add)
                rz = smallp.tile([128, 4], F32, name="rz", tag="rz")
                nc.vector.reciprocal(rz[:], z[:])
                smtau = smallp.tile([128, 4, NK], F32, name="smtau", tag="smtau")
                nc.vector.tensor_tensor(
                    out=smtau[:], in0=e[:].rearrange("p (i j) -> p i j", i=4),
                    in1=rz[:].unsqueeze(2).broadcast_to([128, 4, NK]),
                    op=ALU.mult)
                for i4 in range(4):
                    nc.sync.dma_start(smW[tau][:, i4], smtau[:, i4, :])

            # ---------------- main pipeline ----------------

            # cells chains interleaved with con1 groups at lag 2, so the PE
            # never head-of-line blocks on a chain's ACT/GP/DVE pipeline.
            def stretch(pi, tau):
                # cells pairs 0..4; the previous pair's con1 groups are woven
                # in as two 3-group (12-MM) fragments per pair.
                for s in range(6):
                    fl = None
                    if s >= 1:
                        ka, kb = 2 * (s - 1), 2 * (s - 1) + 1
                        fl = [
                            (lambda ka=ka: emit_con1_k(tau, ka)),
                            (lambda kb=kb: emit_con1_k(tau, kb)),
                        ]
                    if s < 5:
                        emit_cells_pair(pi, s, fillers=fl)
                    elif fl:
                        for f in fl:
                            f()

            def emit_S_tail(tau):
                # scramble read-back + weighted-sum + output; emitted a full
                # phase after emit_XS_head so the DRAM round trip is hidden.
                wa3t = smallp.tile([128, 4 * NK], F32, name="wa3t", tag="wa3t")
                for i4 in range(4):
                    nc.sync.dma_start(wa3t[:, i4 * NK:(i4 + 1) * NK],
                                      smR[tau][:, i4])
                prt = smallp.tile([128, 4], F32, name="prt", tag="prt")
                for i4 in range(4):
                    rt = tau * 4 + i4
                    csc = smallp.tile([128, NK], F32, name="csc", tag="csc")
                    cat = smallp.tile([128, 1], F32, name="cat", tag="cat")
                    nc.vector.scalar_tensor_tensor(
                        csc[:], srow[rt][:], 0.0,
                        wa3t[:, i4 * NK:(i4 + 1) * NK],
                        ALU.add, ALU.mult, accum_out=cat[:])
                    nc.vector.scalar_tensor_tensor(
                        prt[:, i4:i4 + 1], cat[:], 0.0,
                        xrt[tau][:, i4, 10:11], ALU.add, ALU.add)
                nc.sync.dma_start(predsW[tau], prt[:])

            # XS_head right after its fc1 (cheap: ~6 ops); S_tail a full
            # phase later so the smW->smR DRAM round trip never stalls any
            # engine queue (>3.4us PE-idle would re-throttle the clock gate).
            stretch(0, 0)
            emit_fc1(0)
            emit_XS_head(0)
            stretch(1, 1)
            emit_S_tail(0)
            emit_fc1(1)
            emit_XS_head(1)
            for k in range(NK):
                emit_con1_k(2, k)
            emit_S_tail(1)
            emit_fc1(2)
            emit_XS_head(2)
            stretch(2, 3)
            emit_S_tail(2)
            emit_fc1(3)
            emit_XS_head(3)
            emit_S_tail(3)

    nc.compile()
    nc.m = get_hw_module(nc.m)
    return nc


def _get_nc():
    if "nc" not in _BUILD_CACHE:
        _BUILD_CACHE["nc"] = _build()
    return _BUILD_CACHE["nc"]


def _softmax(x):
    e = np.exp(x - x.max())
    return e / e.sum()


def prepare_inputs(local_inputs, labels, extras, DisM, AngleM,
                   Wih, b_ih, b_hh, Wt, bt_ih, bt_hh,
                   wp, bp, F1, b1, F2, b2, ff, bff,
                   fuse1, biasf, Wout, biasout, a):
    """Host-side sharding + layout prep. Returns (in_maps, scalars)."""
    f = np.asarray
    local_inputs, labels, extras = f(local_inputs), f(labels), f(extras)
    aa = float(f(a)[0])
    wA = _softmax(f(DisM).astype(np.float64)).astype(np.float32)

    kept = np.r_[0:300, 600:900, 900:1200]      # gates i, g, o (f is dead)
    b_cells = (f(b_ih) + f(b_hh))[:, kept]      # [10, 900]
    bt = (f(bt_ih) + f(bt_hh))[kept]            # [900]

    # wihT[k]: [12, 900] = [Wih[k,kept,:].T ; bias row]
    wihT = np.zeros((NK, 12, 900), np.float32)
    for k in range(NK):
        wihT[k, :11] = f(Wih)[k][kept, :].T
        wihT[k, 11] = b_cells[k]
    wtT = np.zeros((5, 900), np.float32)
    wtT[:4] = f(Wt)[kept, :].T
    wtT[4] = bt

    # con1 weights x8 in fp8 (the /8 is refolded into f1A below)
    wp8 = (8.0 * f(wp).T).astype(np.float32)    # [600, 300]
    I8 = np.zeros((600, 300), np.float32)
    I8[:300] = 8.0 * np.eye(300, dtype=np.float32)

    def pack_con1(w):
        Dc = np.zeros((128, 2, 320), np.float32)
        Dc[:, :, :300] = w[0:256].reshape(2, 128, 300).transpose(1, 0, 2)
        Dc = Dc.reshape(128, 640)
        Dl = np.zeros((128, 2, 320), np.float32)
        Dl[:, :, :300] = w[300:556].reshape(2, 128, 300).transpose(1, 0, 2)
        Dl = Dl.reshape(128, 640)
        Tc = np.zeros((108, 300), np.float32)
        Tc[0:44] = w[256:300]
        Tc[64:108] = w[256:300]
        Tl = np.zeros((108, 300), np.float32)
        Tl[0:44] = w[556:600]
        Tl[64:108] = w[556:600]
        return (Dc.astype(NPF8), Dl.astype(NPF8),
                Tc.astype(NPF8), Tl.astype(NPF8))

    wpN = pack_con1(wp8)
    wp0 = pack_con1(I8)

    v1 = aa * (f(fuse1) @ f(Wout))[:, 0]        # [300]
    f1A = np.zeros((300, 201), np.float32)
    f1A[:, 0] = v1 / 8.0
    f1A[:, 1:] = f(F1)[:300] / 8.0
    f1B = np.zeros((301, 201), np.float32)
    f1B[:300, 1:] = f(F1)[300:]
    f1B[300, 1:] = f(b1)          # ones-row bias (htar c2 row 44)
    f2full = np.broadcast_to(f(F2)[:200, 0][None, :], (128, 200)).copy()

    ffW = (f(Wout)[:, 0] @ f(ff))               # [17]
    W3 = (1.0 - aa) * np.outer(ffW, wA)         # [17, 10]

    f2_8 = float(f(F2)[200, 0])
    f2a = float(f(F2)[201, 0]) / 360.0
    b2s = float(f(b2)[0])
    c0 = ((1.0 - aa) * float(f(Wout)[:, 0] @ f(bff)[:, 0])
          + aa * float(f(biasf) @ f(Wout)[:, 0])
          + float(f(biasout)[0]))

    bpcol = np.zeros((128, 3), np.float32)
    for oc, (ooff, osz) in enumerate(HC):
        bpcol[:osz, oc] = 8.0 * f(bp)[ooff:ooff + osz, 0]

    in_maps = []
    for cix in range(NCORES):
        t0 = cix * TLOC
        # x block for cells: t0-1 .. t0+3 (zeros for t=-1 on core 0)
        xblk = np.zeros((5, B, 28, NK), np.float32)
        lo = t0 - 1
        for jj in range(5):
            t = lo + jj
            if 0 <= t < T:
                xblk[jj] = local_inputs[:, t]
        xT = np.zeros((NK, 12, 5 * B), np.float32)
        # xT[k, i, taup*512+b] = xblk[taup, b, i, k]
        xT[:, :11, :] = xblk[:, :, :11, :].transpose(3, 2, 0, 1).reshape(NK, 11, 5 * B)
        xT[:, 11, :] = 1.0

        xloc = local_inputs[:, t0:t0 + TLOC]            # [B, 4, 28, 10]
        # host-fold the wdyn extras and the fusiondis term:
        #   wext = F2[200]*fl8 + (F2[201]/360)*|fl10 - Angle| + b2
        #   fusc = xfus . W3 + c0
        fl8 = xloc[:, :, 8, :].transpose(1, 0, 2).reshape(R, NK)
        fl10 = xloc[:, :, 10, :].transpose(1, 0, 2).reshape(R, NK)
        xfus = xloc[:, :, 11:, :].transpose(1, 0, 2, 3).reshape(R, 170)
        xrow = np.empty((R, 11), np.float32)
        xrow[:, 0:10] = (f2_8 * fl8
                         + f2a * np.abs(fl10 - f(AngleM)[None, :]) + b2s)
        xrow[:, 10] = xfus @ W3.reshape(-1) + c0

        exT = np.ones((5, R), np.float32)
        exT[:4] = f(extras)[:, t0:t0 + TLOC, :4, 0].transpose(2, 1, 0).reshape(4, R)

        core0 = cix == 0
        wpz = wp0 if core0 else wpN
        rl = np.zeros((128, 2), np.float32)
        if core0:
            rl[:, 0] = -1e30
        in_maps.append({
            "xT": xT.astype(NPBF),
            "xrow": xrow,
            "exT": exT.astype(NPBF),
            "wihT": wihT.astype(NPBF),
            "wtT": wtT.astype(NPBF),
            "wpDc": wpN[0], "wpDl": wpN[1], "wpTc": wpN[2], "wpTl": wpN[3],
            "wpDc0": wpz[0], "wpDl0": wpz[1], "wpTc0": wpz[2], "wpTl0": wpz[3],
            "bpn": bpcol,
            "bp0": (np.zeros((128, 3), np.float32) if core0 else bpcol),
            "reluL": rl,
            "f1A": f1A.astype(NPBF),
            "f1B": f1B.astype(NPBF),
            "f2full": f2full,
        })
    return in_maps


def kernel(local_inputs, labels, extras, DisM, AngleM,
           Wih, b_ih, b_hh, Wt, bt_ih, bt_hh,
           wp, bp, F1, b1, F2, b2, ff, bff,
           fuse1, biasf, Wout, biasout, a, _trace=False, _tmpdir=None):
    in_maps = prepare_inputs(
        local_inputs, labels, extras, DisM, AngleM,
        Wih, b_ih, b_hh, Wt, bt_ih, bt_hh, wp, bp, F1, b1, F2, b2,
        ff, bff, fuse1, biasf, Wout, biasout, a)
    nc = _get_nc()
    res = bass_utils.run_bass_kernel_spmd(
        nc, in_maps, core_ids=list(range(NCORES)), trace=_trace, tmpdir=_tmpdir)

    preds = np.empty((T, B, 1), np.float32)
    for cix in range(NCORES):
        out = res.results[cix]["preds"].reshape(TLOC, B)
        preds[cix * TLOC:(cix + 1) * TLOC, :, 0] = out

    labels_r = np.ascontiguousarray(
        np.transpose(np.asarray(labels), (1, 0, 2, 3)).reshape(T, B, 1))
    kernel._last_result = res
    return preds, labels_r

